# revision 32
# baseline (speedup 1.0000x reference)
"""Bass/Tile Trainium2 kernel for nn_Attention_14620068676191.

Math (per batch element b, data-parallel over 8 cores):
    q = x @ Wq^T ; k = x @ Wk^T
    scores = q @ k^T / sqrt(D)  ==  x @ (Wq^T Wk) @ x^T / sqrt(D)
    out = softmax(tanh(scores), axis=-1) @ x

We fold the two projections into M = Wq^T @ Wk (computed host-side), so the
per-core work is
    y   = x @ M                     [S, D]           (bf16 matmuls)
    S^T = x @ y^T                   (bf16 matmuls)
    E   = exp(tanh(S^T / sqrt(D)))  (tanh bounds scores: no max-subtraction)
    out = (E @ [x | 1])[:, :D] / Z  (ones column gives the denominator Z)

The PV (E @ x) stage runs in fp8 e4m3 with DoubleRow perf mode: one matmul
instruction contracts TWO 128-row k-chunks in the same issue time as a bf16
matmul, halving the PV instruction count.  Naive e4m3 weights would blow the
error budget, so the positive softmax weights are mean-shifted before
quantization:
    E = C + E',  E' = E - C quantized to e4m3 (~2x smaller quantization error)
    out*Z = E'8 @ x8 + C * colsum(x)   (rank-1 term; colsum computed exactly
                                        host-side, shipped replicated across
                                        partitions, PSUM-preloaded so the PV
                                        matmuls accumulate on top: start=False)
    Z     = sum_t E'8 + C*S            (ones column + scalar bias on the
                                        reciprocal input)
Measured end-to-end absmax relative error vs the fp32 reference: ~1.6e-2
(HW matches the bit-exact numpy model).

Scheduling notes (from NTFF traces):
  * No on-chip transposes: the host ships x twice - transposed bf16
    ([D,S] partition-major, in four s-block-contiguous slabs) for the scores
    stationary / y moving operands, and untransposed e4m3 ([S,1088] with a
    ones column, 64B-aligned rows - unaligned SBUF offsets cost 13us of
    software-DGE descriptor generation) for PV.
  * Same-engine DMAs transfer serially, different engines race for HBM.
    The critical prefix (M + xT s-block 0: dch singles then pairs) runs in
    lockstep on the gpsimd+sync queues only; everything else follows behind
    on the same two queues so it cannot steal bandwidth from the prefix.
  * y(q0) accumulates dch-outer (4 single-e PSUM tiles + 2 e-pair tiles = all
    8 banks) so the first matmuls need only the first m/xT dch chunk;
    y(q1..3) use the 2-bank e-pair form.
  * The PE p-state ramp is burned off by ~96 tiny F=2 warmup matmuls on a
    memset tile while the first DMAs are in flight.  This matters a LOT:
    without the warmup the whole kernel runs ~20% slower (~50us) - the PE
    clock never fully ramps after a long idle.
  * E' = exp(tanh(.)) - C runs as scalar tanh -> scalar exp (bf16) -> DVE
    subtract-with-fp8-cast; all off the PE critical path.  The PV PSUM tiles
    are allocated + ccs-preloaded up to two blocks early so the preload DVE
    copy is never on the PV block's critical path.
  * Output is stored bf16 (host upcasts); the last block's two stores issue
    from the sync queue (gpsimd is draining by then; the scalar engine's
    descriptor generation is ~4x slower).
"""

from contextlib import ExitStack

import ml_dtypes
import numpy as np

import concourse.bass as bass
import concourse.tile as tile
from concourse import bacc, mybir
from concourse.bass import ds, ts
from concourse.bass_utils import run_bass_kernel_spmd

S, B, D = 2048, 8, 1024
P = 128
NS, ND = S // P, D // P  # 16, 8
NB = 512                 # matmul moving-operand block (one PSUM bank fp32)
NQ = S // NB             # 4 s-blocks
DXP = 1088               # x8 row padded to 64B multiple; col D = 1.0, rest 0
F32, BF16 = mybir.dt.float32, mybir.dt.bfloat16
FP8 = mybir.dt.float8e4
AF = mybir.ActivationFunctionType
ALU = mybir.AluOpType
DR = mybir.MatmulPerfMode.DoubleRow
ISCALE = float(D) ** -0.5
C_SHIFT = 1.0            # softmax-weight mean shift (E' = E - C before e4m3)
N_WARM = 96            # p-state warmup matmuls (F=2) during the input DMA
                         # (~55ns each: completion-chained through the PSUM
                         # ring; sized to end right as the first DMAs land)

N_CORES = 8


def _emit(ctx: ExitStack, tc: tile.TileContext, xt_d, m_d, x8_d, cs_d, o_d):
    nc = tc.nc

    pool_xt = ctx.enter_context(tc.tile_pool(name="xt", bufs=1))
    pool_m = ctx.enter_context(tc.tile_pool(name="mw", bufs=1))
    pool_x8 = ctx.enter_context(tc.tile_pool(name="x8", bufs=1))
    pool_cs = ctx.enter_context(tc.tile_pool(name="cs", bufs=1))
    pool_yt = ctx.enter_context(tc.tile_pool(name="yt", bufs=1))
    pool_wm = ctx.enter_context(tc.tile_pool(name="wm", bufs=1))
    pool_big = ctx.enter_context(tc.tile_pool(name="big", bufs=2))
    pool_tt = ctx.enter_context(tc.tile_pool(name="tt", bufs=3))
    pool_osb = ctx.enter_context(tc.tile_pool(name="osb", bufs=3))
    pool_rz = ctx.enter_context(tc.tile_pool(name="rz", bufs=3))
    psum_mm = ctx.enter_context(tc.tile_pool(name="pmm", bufs=4, space="PSUM"))
    psum_pv = ctx.enter_context(tc.tile_pool(name="ppv", bufs=2, space="PSUM"))

    xtb = pool_xt.tile([P, NQ, ND, NB], BF16)  # xtb[p,q,j,s'] = x[q*NB+s', j*P+p]
    m_bf = pool_m.tile([P, ND, D], BF16)       # m_bf[p, j, e] = M[j*P+p, e]
    x8 = pool_x8.tile([P, NS, DXP], FP8)       # x8[p, i, d]  = x[i*P+p, d]
    ccs = pool_cs.tile([P, 2, NB], F32)        # C*colsum(x), replicated rows
    yT = pool_yt.tile([P, ND, S], BF16)        # yT[p, j, s]  = y[s, j*P+p]
    warm = pool_wm.tile([P, P], BF16)

    # ---- input DMAs.  Two queues, strict priority order within each; the
    # same-engine serialization keeps later transfers from stealing HBM
    # bandwidth from the y(q0) prefix (m/xtb0 dch-pairs, in lockstep).
    xt_r = xt_d.rearrange("(qq p j) s -> p qq j s", qq=NQ, p=P)
    m_r = m_d.rearrange("(p j) e -> p j e", p=P)
    x8_r = x8_d.rearrange("(p i) d -> p i d", p=P)
    cs_r = cs_d.rearrange("p (h d) -> p h d", h=2)
    nc.gpsimd.memset(warm, 0.0)
    # dch 0 and 1 land as singles so y(q0) can start ~1.5us earlier
    for jj in range(2):
        nc.gpsimd.dma_start(m_bf[:, jj : jj + 1], m_r[:, jj : jj + 1])
        nc.sync.dma_start(xtb[:, 0, jj : jj + 1], xt_r[:, 0, jj : jj + 1])
    for jj in range(2, ND, 2):
        nc.gpsimd.dma_start(m_bf[:, jj : jj + 2], m_r[:, jj : jj + 2])
        nc.sync.dma_start(xtb[:, 0, jj : jj + 2], xt_r[:, 0, jj : jj + 2])
    nc.sync.dma_start(xtb[:, 1], xt_r[:, 1])
    nc.gpsimd.dma_start(xtb[:, 2], xt_r[:, 2])
    nc.sync.dma_start(xtb[:, 3], xt_r[:, 3])
    nc.gpsimd.dma_start(x8[:, 0:8], x8_r[:, 0:8])
    nc.sync.dma_start(x8[:, 8:16], x8_r[:, 8:16])
    nc.gpsimd.dma_start(ccs, cs_r)

    # ---- PE p-state warmup: tiny matmuls on a memset tile while DMAs fly.
    def warmup(n):
        for i in range(n):
            pw = psum_mm.tile([P, 2], F32, tag="mm", name="pw")
            nc.tensor.matmul(pw, warm, warm[:, 0:2], start=True, stop=True)

    warmup(N_WARM)

    # ---- y^T[e, s] = sum_d M[d, e] * x[s, d] ------------------------------
    # q0 runs dch-outer so each matmul needs only the dch-pair chunks that
    # have already landed: e0..3 in four single-e 1-bank tiles, e4..7 in two
    # 2-bank e-pair tiles (all 8 PSUM banks).
    ps_s = [psum_mm.tile([P, NB], F32, tag="mm", name=f"ps_s{i}") for i in range(4)]
    ps_p = [psum_pv.tile([P, 2, NB], F32, tag="po", name=f"ps_p{i}") for i in range(2)]
    for dch in range(ND):
        st_, sp_ = dch == 0, dch == ND - 1
        mv = xtb[:, 0, dch]
        for e in range(4):
            nc.tensor.matmul(ps_s[e], m_bf[:, dch, ts(e, P)], mv, start=st_, stop=sp_)
        for h in range(2):
            e = 4 + 2 * h
            nc.tensor.matmul(ps_p[h][:, 0], m_bf[:, dch, ts(e, P)], mv, start=st_, stop=sp_)
            nc.tensor.matmul(ps_p[h][:, 1], m_bf[:, dch, ts(e + 1, P)], mv, start=st_, stop=sp_)
    for e in range(4):
        nc.vector.tensor_copy(yT[:, e, 0:NB], ps_s[e])
    for h in range(2):
        nc.vector.tensor_copy(yT[:, 4 + 2 * h : 6 + 2 * h, 0:NB], ps_p[h])

    def y_block(q):
        for e in range(0, ND, 2):
            ps = psum_pv.tile([P, 2, NB], F32, tag="po")
            for dch in range(ND):
                st_, sp_ = dch == 0, dch == ND - 1
                nc.tensor.matmul(
                    ps[:, 0], m_bf[:, dch, ts(e, P)], xtb[:, q, dch],
                    start=st_, stop=sp_,
                )
                nc.tensor.matmul(
                    ps[:, 1], m_bf[:, dch, ts(e + 1, P)], xtb[:, q, dch],
                    start=st_, stop=sp_,
                )
            nc.vector.tensor_copy(yT[:, e : e + 2, ts(q, NB)], ps)

    for q in range(1, NQ):
        y_block(q)

    # ---- per s-block: scores^T -> E' = exp(tanh)-C (fp8) -> PV -> store -----
    def alloc_po(k):
        # allocate + ccs-preload a PV PSUM tile ahead of its block so the
        # DVE copy is never on the PV block's critical path
        po = psum_pv.tile([P, 2, NB], F32, tag="po", name=f"po{k}")
        nc.vector.tensor_copy(po, ccs)
        return po

    for q in range(NQ):
        at8 = pool_big.tile([P, NS, NB], FP8, tag="big")
        po_next = [alloc_po(0), alloc_po(1)]
        for t_i in range(NS):
            ps = psum_mm.tile([P, NB], F32, tag="mm")
            for e in range(ND):
                nc.tensor.matmul(
                    ps,
                    xtb[:, t_i // 4, e, ts(t_i % 4, P)],
                    yT[:, e, ts(q, NB)],
                    start=(e == 0),
                    stop=(e == ND - 1),
                )
            tt = pool_tt.tile([P, NB], BF16, tag="tt")
            nc.scalar.activation(tt, ps, AF.Tanh, scale=ISCALE)
            nc.scalar.activation(tt, tt, AF.Exp)
            nc.vector.tensor_scalar(at8[:, t_i], tt, C_SHIFT, None, ALU.subtract)
        for ss in range(NB // P):
            st = q * (NB // P) + ss
            po = po_next[ss]
            pz = psum_mm.tile([P, 2], F32, tag="mm")
            osb0 = pool_osb.tile([P, NB], BF16, tag="osb0")
            osb1 = pool_osb.tile([P, NB], BF16, tag="osb1")
            zs = pool_rz.tile([P, 1], F32, tag="rz")
            r = pool_rz.tile([P, 1], F32, tag="rz")
            if st == S // P - 1:
                # last block: run the po0+pz chains first, then po1, so the
                # po0 normalize and its store overlap the po1 matmuls and
                # only the po1 half remains on the kernel tail
                for tp in range(NS // 2):
                    lw = at8[:, 2 * tp : 2 * tp + 2, ts(ss, P)]
                    xp = x8[:, 2 * tp : 2 * tp + 2]
                    first, last = tp == 0, tp == NS // 2 - 1
                    nc.tensor.matmul(po[:, 0], lw, xp[:, :, 0:NB], start=False, stop=last, perf_mode=DR)
                    nc.tensor.matmul(pz, lw, xp[:, :, D : D + 2], start=first, stop=last, perf_mode=DR)
                nc.vector.tensor_scalar(zs, pz[:, 0:1], C_SHIFT * S, None, ALU.add)
                nc.vector.reciprocal(r, zs)
                nc.scalar.mul(osb0, po[:, 0], r)
                nc.sync.dma_start(o_d[ts(st, P), 0:NB], osb0)
                for tp in range(NS // 2):
                    lw = at8[:, 2 * tp : 2 * tp + 2, ts(ss, P)]
                    xp = x8[:, 2 * tp : 2 * tp + 2]
                    nc.tensor.matmul(po[:, 1], lw, xp[:, :, NB:D], start=False, stop=(tp == NS // 2 - 1), perf_mode=DR)
                nc.vector.tensor_scalar_mul(osb1, po[:, 1], r)
                nc.sync.dma_start(o_d[ts(st, P), NB:D], osb1)
            else:
                for tp in range(NS // 2):
                    lw = at8[:, 2 * tp : 2 * tp + 2, ts(ss, P)]
                    xp = x8[:, 2 * tp : 2 * tp + 2]
                    first, last = tp == 0, tp == NS // 2 - 1
                    if last:
                        # denominator first so the reciprocal can start while
                        # the two output matmuls finish
                        nc.tensor.matmul(pz, lw, xp[:, :, D : D + 2], start=first, stop=last, perf_mode=DR)
                        nc.tensor.matmul(po[:, 0], lw, xp[:, :, 0:NB], start=False, stop=last, perf_mode=DR)
                        nc.tensor.matmul(po[:, 1], lw, xp[:, :, NB:D], start=False, stop=last, perf_mode=DR)
                    else:
                        nc.tensor.matmul(po[:, 0], lw, xp[:, :, 0:NB], start=False, stop=last, perf_mode=DR)
                        nc.tensor.matmul(po[:, 1], lw, xp[:, :, NB:D], start=False, stop=last, perf_mode=DR)
                        nc.tensor.matmul(pz, lw, xp[:, :, D : D + 2], start=first, stop=last, perf_mode=DR)
                nc.vector.tensor_scalar(zs, pz[:, 0:1], C_SHIFT * S, None, ALU.add)
                nc.vector.reciprocal(r, zs)
                # normalize the two halves on different engines in parallel
                nc.scalar.mul(osb0, po[:, 0], r)
                nc.vector.tensor_scalar_mul(osb1, po[:, 1], r)
                nc.gpsimd.dma_start(o_d[ts(st, P), 0:NB], osb0)
                nc.gpsimd.dma_start(o_d[ts(st, P), NB:D], osb1)
            if ss + 2 < NB // P:
                po_next.append(alloc_po(ss + 2))


def build_program() -> bass.Bass:
    nc = bacc.Bacc("TRN2", target_bir_lowering=False, debug=False)
    xt_d = nc.declare_dram_parameter("xt", [NQ * D, NB], BF16, isOutput=False)
    m_d = nc.declare_dram_parameter("m", [D, D], BF16, isOutput=False)
    x8_d = nc.declare_dram_parameter("x8", [S, DXP], FP8, isOutput=False)
    cs_d = nc.declare_dram_parameter("cs", [P, D], F32, isOutput=False)
    o_d = nc.declare_dram_parameter("out", [S, D], BF16, isOutput=True)
    with tile.TileContext(nc) as tc:
        with ExitStack() as ctx:
            _emit(ctx, tc, xt_d.ap(), m_d.ap(), x8_d.ap(), cs_d.ap(), o_d.ap())
    nc.compile()
    return nc


_CACHE: dict = {}


def _get_program() -> bass.Bass:
    if "nc" not in _CACHE:
        _CACHE["nc"] = build_program()
    return _CACHE["nc"]


def _prep_core_inputs(xb: np.ndarray):
    """Per-core host prep: xb is [S, D] fp32."""
    # xT in four s-block slabs, partition-major:
    # row q*D + p*ND + j holds x[q*NB : (q+1)*NB, j*P + p]
    xt = np.ascontiguousarray(
        xb.T.reshape(ND, P, NQ, NB).transpose(2, 1, 0, 3).reshape(NQ * D, NB)
    ).astype(ml_dtypes.bfloat16)
    # x8 with pad cols, partition-major: row p*NS + i holds x[i*P + p, :]+pad
    x8 = np.zeros((S, DXP), dtype=ml_dtypes.float8_e4m3)
    x8[:, :D] = xb.astype(ml_dtypes.float8_e4m3)[
        np.arange(S).reshape(NS, P).T.reshape(S)
    ]
    x8[:, D] = 1.0
    # C * colsum(x), exact (fp64 accumulate), replicated across partitions
    cs_row = (C_SHIFT * xb.sum(0, dtype=np.float64)).astype(np.float32)
    cs = np.ascontiguousarray(np.broadcast_to(cs_row, (P, D)))
    return {"xt": xt, "x8": x8, "cs": cs}


def run(x, Wq, Wk, trace: bool = False):
    """Run on 8 NeuronCores (batch-parallel). Returns (out, BassKernelResults)."""
    x = np.asarray(x, dtype=np.float32)
    wq = np.asarray(Wq, dtype=np.float32)
    wk = np.asarray(Wk, dtype=np.float32)
    m_full = (wq.T @ wk).astype(ml_dtypes.bfloat16)
    # rows reordered (j*128+p) -> (p*8+j) so each SBUF partition's 8 rows are
    # contiguous in DRAM
    m_perm = np.ascontiguousarray(
        m_full.reshape(ND, P, D).transpose(1, 0, 2).reshape(D, D)
    )
    nc = _get_program()
    in_maps = []
    for b in range(N_CORES):
        im = _prep_core_inputs(x[:, b, :])
        im["m"] = m_perm
        in_maps.append(im)
    res = run_bass_kernel_spmd(nc, in_maps, list(range(N_CORES)), trace=trace)
    out = np.stack(
        [res.results[b]["out"].astype(np.float32) for b in range(N_CORES)], axis=1
    )
    return out, res


def kernel(x, Wq, Wk):
    out, _ = run(x, Wq, Wk)
    return out
